# revision 9
# baseline (speedup 1.0000x reference)
"""Trainium2 Bass kernel for nn_BinaryLayer (logic-gate network).

Computes: out[b, o] = OR_t AND_a x_in[b, weights[o, t, a]]
where x_in = [const_true | (x != 0) | ~(x != 0)]  (width 1 + 2*784 = 1569),
plus an or-mask: an (o, t) gate whose 16 indices are all 0 is forced False.

Strategy (8 NeuronCores, tensor-parallel over OUT: 128 outs per core):
  The AND over a gate's 16 terms equals (sum of its selected bits == 16).
  Each selected bit is const-1, x[f], or 1-x[f], so the sum is affine in x:
      S[b, o, t] = base[o,t] + sum_f M[f, (o,t)] * x[b, f]
  with integer M (<=16 nonzeros per column, |M| <= 16) and integer base
  (masked gates get M column = base = 0, so S = 0 < 16).
  Since S <= 16 always,  OR_t (S_t == 16)  ==  (max_t S_t == 16).

  Per core pipeline (per 512-batch chunk):
    1. PE: S-tiles [128 o, 512 b], one per t (32 tiles), as fp8(e4m3)
       DoubleRow matmuls over 1024 padded features (4 passes of 256; exact:
       all values are small integers; accumulation is fp32 PSUM).
    2. DVE: running elementwise max over the 32 S-tiles (copy, then 31 maxes).
    3. ACT: out u8 = relu(max - 15) in {0,1}; DMA out [128 o, 1024 b].
"""

import numpy as np

B, F = 1024, 784
OUT, OR_T, AND_T = 1024, 32, 16
N_CORES = 8
O_LOC = OUT // N_CORES  # 128 outs per core
OT_LOC = O_LOC * OR_T  # 4096 (o,t) columns per core
NK = 4  # DoubleRow k-passes over 1024 padded features (256 each)
NB = 2  # batch chunks
BCH = B // NB  # 512

_cache = {}


def _build(reps=1, loop=False, inner=1):
    import concourse.mybir as mybir
    import concourse.tile as tile
    from concourse.bacc import Bacc

    f32 = mybir.dt.float32
    u8 = mybir.dt.uint8
    f8 = mybir.dt.float8e4
    Act = mybir.ActivationFunctionType
    Alu = mybir.AluOpType
    DR = mybir.MatmulPerfMode.DoubleRow

    nc = Bacc("TRN2", target_bir_lowering=False, debug=False, num_devices=N_CORES)
    xq_t = nc.dram_tensor("xq", [128, NK, 2, B], u8, kind="ExternalInput")
    mq_t = nc.dram_tensor("mq", [128, NK, 2, OT_LOC], u8, kind="ExternalInput")
    out_t = nc.dram_tensor("out", [128, B], u8, kind="ExternalOutput")

    with tile.TileContext(nc) as tc:
        with (
            tc.tile_pool(name="main", bufs=1) as pool,
            tc.tile_pool(name="runp", bufs=2) as rpool,
            tc.tile_pool(name="outp", bufs=2) as opool,
            tc.tile_pool(name="pp1", bufs=8, space="PSUM") as pp1,
        ):
            xq = pool.tile([128, NK, 2, B], u8)
            mq = pool.tile([128, NK, 2, OT_LOC], u8)
            # split DMAs: xq per k-pass, mq per ot-range (512 cols = 4 tiles)
            # so the first matmuls can start before all input lands.
            for k in range(NK):
                nc.sync.dma_start(xq[:, k], xq_t.ap()[:, k])
            for r in range(8):
                nc.sync.dma_start(
                    mq[:, :, :, 512 * r : 512 * (r + 1)],
                    mq_t.ap()[:, :, :, 512 * r : 512 * (r + 1)],
                )
            xqf = xq[:].bitcast(f8)
            mqf = mq[:].bitcast(f8)
            bias15 = pool.tile([128, 1], f32)
            nc.vector.memset(bias15[:], -15.0)

            def body(_i=None):
                for bc in range(NB):
                    run = rpool.tile([128, BCH], f32, tag="run")
                    for t in range(OR_T):
                        ps1 = pp1.tile([128, BCH], f32, tag="ps1")
                        for k in range(NK):
                            nc.tensor.matmul(
                                out=ps1[:],
                                lhsT=mqf[:, k, :, 128 * t : 128 * (t + 1)],
                                rhs=xqf[:, k, :, BCH * bc : BCH * (bc + 1)],
                                start=(k == 0),
                                stop=(k == NK - 1),
                                perf_mode=DR,
                                skip_group_check=True,
                            )
                        if t == 0:
                            nc.vector.tensor_copy(out=run[:], in_=ps1[:])
                        else:
                            nc.vector.tensor_tensor(
                                out=run[:], in0=run[:], in1=ps1[:], op=Alu.max
                            )
                    res = opool.tile([128, BCH], u8, tag="res")
                    nc.scalar.activation(res[:], run[:], Act.Relu, bias=bias15[:])
                    nc.sync.dma_start(
                        out_t.ap()[:, BCH * bc : BCH * (bc + 1)], res[:]
                    )

            if loop and reps > 1:
                with tc.For_i(0, reps):
                    for _ in range(inner):
                        body()
            else:
                for _ in range(reps):
                    body()
    nc.compile()
    return nc


def _host_inputs(x, weights):
    import ml_dtypes

    f8np = ml_dtypes.float8_e4m3

    x = np.ascontiguousarray(np.asarray(x, dtype=np.float32))
    w = np.asarray(weights).astype(np.int64).reshape(OUT * OR_T, AND_T)

    # S[b, ot] = base[ot] + x[b] @ M[:, ot]; padded feature 784 = const 1.
    Mt = np.zeros((OUT * OR_T, 1024), np.float32)  # [ot, fpad]
    ot_ids = np.repeat(np.arange(OUT * OR_T), AND_T)
    v = w.ravel()
    neg = v >= (1 + F)
    f_idx = np.where(neg, v - 1 - F, v - 1)
    sgn = np.where(neg, -1.0, 1.0).astype(np.float32)
    sel = v >= 1
    np.add.at(Mt, (ot_ids[sel], f_idx[sel]), sgn[sel])
    base = (v == 0).reshape(-1, AND_T).sum(1) + neg.reshape(-1, AND_T).sum(1)
    Mt[:, F] = base.astype(np.float32)
    allzero = (w == 0).all(1)
    Mt[allzero] = 0.0

    # per-core column order: tile t holds columns (o=0..127, t fixed):
    # col index within core = 128*t + o  ->  ot = (o * OR_T + t)
    Mt = Mt.reshape(N_CORES, O_LOC, OR_T, 1024).transpose(0, 2, 1, 3)
    Mt = Mt.reshape(N_CORES, OT_LOC, 1024)

    # fp8 encodings, feature-major layout: f = 256k + 128j + p
    Mq = Mt.transpose(2, 0, 1).astype(f8np).view(np.uint8)  # [1024f, nc, 4096]
    Mq = Mq.reshape(NK, 2, 128, N_CORES, OT_LOC).transpose(3, 2, 0, 1, 4)
    mq_cores = [np.ascontiguousarray(Mq[cc]) for cc in range(N_CORES)]

    xT = np.zeros((1024, B), np.float32)
    xT[:F] = (x.T != 0).astype(np.float32)
    xT[F] = 1.0
    xq = xT.astype(f8np).view(np.uint8)
    xq = np.ascontiguousarray(xq.reshape(NK, 2, 128, B).transpose(2, 0, 1, 3))

    return xq, mq_cores


def _assemble(results):
    out = np.zeros((B, OUT), dtype=bool)
    for cc in range(N_CORES):
        r = np.ascontiguousarray(results[cc]["out"]).view(np.uint8)
        out[:, O_LOC * cc : O_LOC * (cc + 1)] = (r != 0).T
    return out


def kernel(x, weights):
    from concourse.bass_utils import run_bass_kernel_spmd

    if "nc" not in _cache:
        _cache["nc"] = _build(reps=1)
    nc = _cache["nc"]

    xq, mq_cores = _host_inputs(x, weights)
    in_maps = [{"xq": xq, "mq": mq_cores[cc]} for cc in range(N_CORES)]
    try:
        res = run_bass_kernel_spmd(nc, in_maps, core_ids=list(range(N_CORES)))
    except Exception:
        # transient device/tunnel errors: retry once on a fresh attempt
        res = run_bass_kernel_spmd(nc, in_maps, core_ids=list(range(N_CORES)))
    return _assemble(res.results)
